# revision 32
# baseline (speedup 1.0000x reference)
"""Morphological dilation (max-plus 3x3 depthwise conv) on 8 Trainium2 cores.

out[b,c,y,x] = max_{i,j in 3x3} ( x_pad[b,c,y+i,x+j] + se[c,i,j] ),
x: [16,64,256,256] f32, se: [64,3,3] f32, pad=1 with CVAL=-10000.

Sharding: pure data parallel. Core k takes batches {2k, 2k+1}; the 2*64
(batch,channel) pairs map onto the 128 SBUF partitions, so se[c,i,j] is a
per-partition scalar. Spatial dims live on the free axis.

Engine roles (measured: DVE fp16 tensor_scalar 4x when full-tile/aligned,
2x otherwise; tensor_tensor 2x; ACT Identity(x + per-partition bias) 1x at
1.2 GHz; GpSimd tensor ops ~10x below the cost-model roofline -> unused for
compute; PE cannot max; codegen rejects TT/STT on Pool). Per 9-tap block:
DVE does the j=0 adds (tensor_scalar, init + 2 taps) + all 8 maxes
(tensor_tensor); ACT does the 6 j=1/j=2 adds into 5 rotating slot tiles
(tap g -> slot g%5, freed by the fold TT of tap g for tap g+5). The last
`sp` rows of the last two ACT taps go to DVE tensor_scalar to balance
DVE ~= ACT ~= 29.5 us per 22-row block. The hand-scheduled emission order
(init, prev-block deferred fold7+store, d1 f0 d2 f1, ACT taps, remaining
folds) hands ACT its slot-free gates ~5 us apart, which measures as a
gapless DVE steady state; the last fold + store of each block are deferred
into the next block as DVE filler (software pipeline across blocks).

Ramp/DMA findings baked in: x is host-pre-padded to [P,258,258] fp16 so
every load chunk is one contiguous descriptor per partition (strided
256-into-258 loads ran at ~218 GB/s and starved the ramp); loads stay on
ONE serial HWDGE queue because a second concurrent load chain contends for
SBUF ports and slows every DVE/ACT op ~20%; chunk sizes pace arrivals
(~0.58 us/row while computing) ahead of consumption (~1.34 us/row). The
per-block stores rotate over HWDGE queues 1-6 (q7 holds the tiny se load,
q0 the input chain); SWDGE is avoided (Pool-engine desc-gen costs ~1 us +
serial drain at the kernel tail).

Sync-wait budgets are 1 per instruction for the compute/DMA encodings used
here. Cross-engine handoffs rely on Tile auto-deps; instructions that would
carry >1 waits get same-engine Drain spills from a post-pass
(_split_excess_waits). Cheap 1-element gates keep the hot instructions at a
single wait: DVE memsets gw (acc-slot WAR vs the b-2 store) and gx (input
chunk), plus one ACT copy-gate ga per block (input chunk; reads se_t so its
waits are DMA sems only).
"""

import os
import numpy as np

B, C, H, W = 16, 64, 256, 256
NCORES = 8
P = 128  # partitions = (B // NCORES) * C
CVAL = -10000.0
KH = KW = 3

_DTYPE = os.environ.get("DILATION_DTYPE", "f16")
_SP = int(os.environ.get("DILATION_SP", "2"))  # rows of a4/a5 done by DVE
_ROWS = int(os.environ.get("DILATION_ROWS", "22"))  # steady block rows
_NSLOTS = int(os.environ.get("DILATION_NSLOTS", "5"))  # rotating slot tiles
_DEFER = int(os.environ.get("DILATION_DEFER", "1"))  # folds deferred per block

_nc_cache = {}
LAST_RESULTS = None  # BassKernelResults of the most recent run (for profiling)

# instruction name -> forced HWDGE queue index (consulted by the patched
# TileClockTick._assign_tick during scheduling)
_FORCED_HW_QUEUE = {}
_ASSIGN_PATCHED = False

# tap order: g0,g1 = DVE tensor_scalar taps (j=0); g2..g7 = ACT taps.
# tap (0,0) is the acc init (extra, unslotted). Scalar index t = 3*i + j.
_DVE_TAPS = [(1, 0), (2, 0)]
_ACT_TAPS = [(0, 1), (1, 1), (2, 1), (0, 2), (1, 2), (2, 2)]


def _patch_queue_assignment():
    global _ASSIGN_PATCHED
    if _ASSIGN_PATCHED:
        return
    import concourse.tile_sem_assignment as tsa

    orig = tsa.TileClockTick._assign_tick

    def _assign_tick(self, inst):
        forced = _FORCED_HW_QUEUE.get(getattr(inst, "name", None))
        if forced is None:
            return orig(self, inst)
        save = self.next_hw_dma_idx
        self.next_hw_dma_idx = forced
        try:
            return orig(self, inst)
        finally:
            self.next_hw_dma_idx = save

    tsa.TileClockTick._assign_tick = _assign_tick
    _ASSIGN_PATCHED = True


def _split_excess_waits(nc, mybir, max_waits: int = 1):
    """Walrus's per-encoding sync-wait slots are scarce (1 for most ops used
    here). Hoist all but `max_waits` waits of any instruction onto freshly
    inserted same-engine Drain instructions placed right before it."""
    n = 0
    for bb in nc.main_func.blocks:
        insts = bb.instructions
        i = 0
        while i < len(insts):
            ins = insts[i]
            si = ins.sync_info
            if si is not None and len(si.on_wait) > max_waits:
                waits = list(si.on_wait)
                keep = waits[-max_waits:]
                spill = waits[:-max_waits]
                new_insts = []
                for w in spill:
                    d = mybir.InstDrain(name=f"wsplit-{n}", ins=[], outs=[])
                    n += 1
                    d.engine = ins.engine
                    d.sync_info = mybir.SyncInfo(on_wait=[w], on_update=[])
                    new_insts.append(d)
                ins.sync_info = mybir.SyncInfo(
                    on_wait=keep, on_update=list(si.on_update)
                )
                insts[i:i] = new_insts
                i += len(new_insts)
            i += 1
        bb.instructions = insts


def _build(
    dtype_tag: str,
    h: int = H,
    sp: int = _SP,
    rows_s: int = _ROWS,
    nslots: int = _NSLOTS,
    defer: int = _DEFER,
):
    import concourse.bass as bass
    import concourse.mybir as mybir
    from concourse.tile import TileContext, add_dep_helper

    _patch_queue_assignment()
    _FORCED_HW_QUEUE.clear()

    assert dtype_tag == "f16", "layout is fp16-only"
    dt = mybir.dt.float16
    f32 = mybir.dt.float32
    add = mybir.AluOpType.add
    vmax = mybir.AluOpType.max
    ident = mybir.ActivationFunctionType.Identity
    fcopy = mybir.ActivationFunctionType.Copy

    nc = bass.Bass(trn_type="TRN2", num_swdge_queues=4)
    # x arrives host-pre-padded to [P, h+2, W+2] with CVAL borders, so every
    # DMA chunk is one contiguous descriptor per partition (the old
    # 256-cols-into-258-pitch loads ran at ~218 GB/s, gating the ramp).
    x_d = nc.declare_dram_parameter("x", [P, h + 2, W + 2], dt, isOutput=False)
    se_d = nc.declare_dram_parameter("sep", [P, KH * KW], f32, isOutput=False)
    out_d = nc.declare_dram_parameter("out", [P, h, W], dt, isOutput=True)

    # Early blocks taper up (8,12,18) so consumption ramps with the serial
    # load chain's contended delivery (~0.58us/row + 2.2us/chunk issue
    # dead-time); matching just-in-time chunk sizes. Loads must stay on ONE
    # serial queue — a concurrent second load chain contends for SBUF write
    # ports and slows every DVE/ACT op ~20%. Small last blocks drain fast.
    if rows_s == 22:
        blocks = [8] + [22] * 10 + [20, 8]
        loads = [10, 28, 22, 38, 60, 60, 40]
    else:
        nsteady = (h - 16) // rows_s
        tail = h - 16 - nsteady * rows_s
        blocks = [8] + [rows_s] * nsteady + ([tail] if tail else []) + [8]
        loads = [10, 28, 60, 60, 60, 40]
    assert sum(blocks) == h and all(b % 2 == 0 and b > 0 for b in blocks)
    maxrows = max(blocks)
    # SBUF: xt + (2 acc + nslots) block tiles must fit in ~208 KiB/partition
    assert (2 + nslots) * maxrows * 2 * W + (h + 2) * (W + 2) * 2 <= 212500
    load_queue = [0] * len(loads)
    assert sum(loads) == h + 2

    with TileContext(nc) as tc:
        with (
            tc.tile_pool(name="const", bufs=1) as cpool,
            tc.tile_pool(name="xp", bufs=1) as xpool,
            tc.tile_pool(name="accp", bufs=2) as apool,
            tc.tile_pool(name="tmpp", bufs=1) as tpool,
        ):
            # se on its own queue so chunk0 starts immediately on q0
            se_t = cpool.tile([P, KH * KW], f32, name="se_t")
            se_dma = nc.sync.dma_start(out=se_t[:], in_=se_d[:])
            _FORCED_HW_QUEUE[se_dma.ins.name] = 7

            # One persistent padded-x tile: xt row t = padded-input row t.
            xt = xpool.tile([P, h + 2, W + 2], dt, name="xt")

            # Chunked contiguous loads; chunks on the same queue chain
            # serially, the two queues run concurrently.
            load_dmas = []
            load_span = []  # (start, top) padded rows per chunk
            y0 = 0
            for rows, lq in zip(loads, load_queue):
                ld = nc.sync.dma_start(
                    out=xt[:, y0 : y0 + rows, :],
                    in_=x_d[:, y0 : y0 + rows, :],
                )
                _FORCED_HW_QUEUE[ld.ins.name] = lq
                load_dmas.append(ld)
                load_span.append((y0, y0 + rows))
                y0 += rows

            # tiny scratch targets for the DVE gates
            dve_scr = cpool.tile([P, 2 * len(blocks)], dt, name="dve_scr")
            act_scr = cpool.tile([P, 2], dt, name="act_scr")

            # nslots rotating slot tiles shared by the 8 non-init taps of each
            # block: global tap g -> slot g%nslots; the fold TT of tap g frees
            # the slot for tap g+nslots (Tile auto-deps enforce the WAR).
            slots = [
                tpool.tile([P, maxrows, W], dt, name=f"slot{i}")
                for i in range(nslots)
            ]

            out_dmas = []
            pending = []  # deferred (fold-emitter, store-emitter) of prev block
            y0 = 0
            for blk, rows in enumerate(blocks):
                # load chunks this block needs: the deepest overlapping
                # chunk on each queue (same-queue chunks chain in order)
                lo, hi = y0, y0 + rows + 2
                need = {}
                for i, (s, t) in enumerate(load_span):
                    if s < hi and t > lo:
                        need[load_queue[i]] = i
                ldis = sorted(need.values())

                acc = apool.tile([P, rows, W], dt, name="acc")
                # DVE-side gates: gw absorbs the store whose acc slot this
                # block reuses, gx the input-chunk wait(s).
                if blk >= 2:
                    gw = nc.vector.memset(dve_scr[:, 2 * blk + 1 : 2 * blk + 2], 0.0)
                    add_dep_helper(gw.ins, out_dmas[blk - 2].ins, reason="acc WAR")
                gx = nc.vector.memset(dve_scr[:, 2 * blk : 2 * blk + 1], 0.0)
                for ldi in ldis:
                    add_dep_helper(gx.ins, load_dmas[ldi].ins, reason="input chunk")
                # ACT-side chunk gate; reads se_t so its waits are DMA sems
                ga = nc.scalar.activation(act_scr[:, 0:1], se_t[:, 0:1], fcopy)
                for ldi in ldis:
                    add_dep_helper(ga.ins, load_dmas[ldi].ins, reason="chunk/ACT")

                # acc init: tap (0,0), aligned full-tile tensor_scalar (4x)
                nc.vector.tensor_scalar(
                    acc[:],
                    xt[:, y0 : y0 + rows, 0:W],
                    se_t[:, 0:1],
                    None,
                    add,
                )

                # 8 non-init taps: g=0,1 are DVE tensor_scalar (j=0, full-tile
                # writes keep 4x); g=2..7 are ACT adds, the last two with an
                # sp-row DVE tail (aligned j=2) to balance the engines.
                spl = sp if rows >= 12 else 0
                gbase = 8 * blk

                def emit_tap(g, y0=y0, rows=rows, spl=spl, gbase=gbase):
                    st = slots[(gbase + g) % nslots]
                    if g < 2:
                        t_i, t_j = _DVE_TAPS[g]
                        sidx = 3 * t_i + t_j
                        nc.vector.tensor_scalar(
                            st[:, 0:rows, :] if rows < maxrows else st[:],
                            xt[:, y0 + t_i : y0 + t_i + rows, t_j : t_j + W],
                            se_t[:, sidx : sidx + 1],
                            None,
                            add,
                        )
                        return
                    k = g - 2
                    t_i, t_j = _ACT_TAPS[k]
                    sidx = 3 * t_i + t_j
                    arows = rows - spl if k >= 4 else rows
                    nc.scalar.activation(
                        st[:, 0:arows, :],
                        xt[:, y0 + t_i : y0 + t_i + arows, t_j : t_j + W],
                        ident,
                        bias=se_t[:, sidx : sidx + 1],
                    )
                    if arows < rows:
                        nc.vector.tensor_scalar(
                            st[:, arows:rows, :],
                            xt[:, y0 + t_i + arows : y0 + t_i + rows, t_j : t_j + W],
                            se_t[:, sidx : sidx + 1],
                            None,
                            add,
                        )

                def emit_fold(g, acc=acc, rows=rows, gbase=gbase):
                    nc.vector.tensor_tensor(
                        acc[:], acc[:], slots[(gbase + g) % nslots][:, 0:rows, :],
                        vmax,
                    )

                def emit_store(blk=blk, acc=acc, y0=y0, rows=rows):
                    # HWDGE queues 1..6 round-robin (7 is the se load); a
                    # queue's prior store finished ~6 block-periods earlier.
                    od = nc.sync.dma_start(
                        out=out_d[:, y0 : y0 + rows, :], in_=acc[:]
                    )
                    _FORCED_HW_QUEUE[od.ins.name] = 1 + (blk % 6)
                    out_dmas.append(od)

                # program order: tap g must follow fold g-nslots (slot reuse);
                # the last `defer` folds (+ store) move into the next block.
                if nslots == 5 and defer == 1:
                    # hand-scheduled zero-stall order: early d1/f0/d2/f1 give
                    # ACT its slot-free gates ~5us apart; prev f7+store land
                    # right after init as DVE filler.
                    for emit in pending:
                        emit()
                    pending = []
                    for step, g in (
                        ("t", 0), ("f", 0), ("t", 1), ("f", 1),
                        ("t", 2), ("t", 3), ("t", 4), ("t", 5), ("t", 6),
                        ("f", 2), ("t", 7), ("f", 3), ("f", 4), ("f", 5),
                        ("f", 6),
                    ):
                        (emit_tap if step == "t" else emit_fold)(g)
                    pending = [lambda f=emit_fold: f(7), emit_store]
                else:
                    n_head = 2  # d1,d2 freed >= 2 folds before the deferred
                    for g in range(n_head):
                        emit_tap(g)
                    for emit in pending:
                        emit()
                    pending = []
                    for g in range(n_head, min(nslots, 8)):
                        emit_tap(g)
                    for g in range(8):
                        emit_fold(g)
                        if g + nslots < 8:
                            emit_tap(g + nslots)
                        if g == 7 - defer:
                            break
                    for g in range(8 - defer, 8):
                        pending.append(lambda g=g, f=emit_fold: f(g))
                    pending.append(emit_store)
                if defer == 0:
                    for emit in pending:
                        emit()
                    pending = []
                y0 += rows
            for emit in pending:
                emit()

    _split_excess_waits(nc, mybir)
    return nc


def _get_nc():
    key = (_DTYPE, _SP)
    if key not in _nc_cache:
        _nc_cache[key] = _build(_DTYPE)
    return _nc_cache[key]


def kernel(x: np.ndarray, se: np.ndarray) -> np.ndarray:
    global LAST_RESULTS
    from concourse.bass_utils import run_bass_kernel_spmd

    np_dt = np.float16 if _DTYPE == "f16" else np.float32
    x = np.asarray(x)
    se = np.asarray(se)
    # host-side CVAL pre-pad -> every device DMA chunk is contiguous
    xs = np.full((NCORES, P, H + 2, W + 2), CVAL, dtype=np_dt)
    xs[:, :, 1 : H + 1, 1 : W + 1] = (
        np.ascontiguousarray(x).reshape(NCORES, P, H, W).astype(np_dt)
    )
    sep = np.ascontiguousarray(
        np.tile(np.asarray(se, np.float32).reshape(C, KH * KW), (P // C, 1))
    )

    nc = _get_nc()
    in_maps = [{"x": xs[k], "sep": sep} for k in range(NCORES)]
    trace = bool(os.environ.get("DILATION_TRACE"))
    kwargs = {}
    if trace:
        kwargs["trace"] = True
        tmpdir = os.environ.get("DILATION_TRACE_DIR")
        if tmpdir:
            kwargs["tmpdir"] = tmpdir
    res = run_bass_kernel_spmd(nc, in_maps, list(range(NCORES)), **kwargs)
    LAST_RESULTS = res
    out = np.stack([res.results[k]["out"] for k in range(NCORES)])
    return out.reshape(B, C, H, W).astype(np.float32)
